# revision 3
# baseline (speedup 1.0000x reference)
"""LSTM-pool kernel for Trainium2, 8-core data-parallel SPMD.

Math (per batch row b):
  x_t = [seq[b,t], seq_e[b,t], seq_t[b,t]]              (A = 384)
  z_t = x_t @ Wi + h_{t-1} @ Wh + bh                    (4F = 512, gates i,f,o,g)
  c_t = sig(f)*c_{t-1} + sig(i)*tanh(g);  h_t = sig(o)*tanh(c_t)
  out = relu([h_T, src] @ W1 + b1) @ W2 + b2

Device layout: transposed (feature on partitions, batch on the free dim).
The input projection u_{t+1} = x_{t+1} @ Wi runs one step ahead of the
recurrence at N=512 (full per-core batch) into a ping-pong PSUM tile
[128, 4 quad, 512 b] (4 banks each); the recurrent Wh matmuls then
accumulate into the same PSUM regions per half-batch (256 rows), so
z = u + Wh h needs no separate add.  Gate math is bf16 end-to-end on
DVE (2x mode), activations on ACT read PSUM directly:
  per half-step: sigmoid over [i|f|o] (768 free), tanh(g), tanh(c).
Two half-batches are staggered so ACT/DVE of one half hide under the
matmuls of the other.
x^T is produced by fp32->bf16 cast-DMA (SWDGE) + SBUF->SBUF xbar transpose.
"""

import sys

sys.path.insert(0, "/opt/trn_rl_repo")

import numpy as np

import concourse.bass as bass
import concourse.mybir as mybir
import concourse.tile as tile
from concourse import bacc
from concourse.bass_utils import run_bass_kernel_spmd

dt = mybir.dt
AF = mybir.ActivationFunctionType

NCORES = 8
BFULL = 4096
B = BFULL // NCORES  # 512 batch rows per core
T = 128
F = 128
A = 384
G = 512  # 4F
TC = 8  # time steps per DMA chunk
NH = B // 2  # half-batch = 256

# z quad order along the PSUM free dim: [i | f | o | g] so one sigmoid op
# covers quads 0..2 and tanh covers quad 3.  Column offsets into Wi/Wh.
QUADS = [("i", 0, 0), ("f", 1, 128), ("o", 2, 384), ("g", 3, 256)]


def build_nc(zero_bias: bool, t_steps: int = T):
    nc = bacc.Bacc("TRN2", target_bir_lowering=False, debug=False, num_devices=NCORES)

    seq = nc.dram_tensor("seq", [B, T, F], dt.float32, kind="ExternalInput")
    seq_e = nc.dram_tensor("seq_e", [B, T, F], dt.float32, kind="ExternalInput")
    seq_t = nc.dram_tensor("seq_t", [B, T, F], dt.float32, kind="ExternalInput")
    src = nc.dram_tensor("src", [B, F], dt.float32, kind="ExternalInput")
    Wi = nc.dram_tensor("Wi", [A, G], dt.float32, kind="ExternalInput")
    Wh = nc.dram_tensor("Wh", [F, G], dt.float32, kind="ExternalInput")
    bh = nc.dram_tensor("bh", [G], dt.float32, kind="ExternalInput")
    W1 = nc.dram_tensor("W1", [2 * F, F], dt.float32, kind="ExternalInput")
    b1 = nc.dram_tensor("b1", [F], dt.float32, kind="ExternalInput")
    W2 = nc.dram_tensor("W2", [F, F], dt.float32, kind="ExternalInput")
    b2 = nc.dram_tensor("b2", [F], dt.float32, kind="ExternalInput")
    outT = nc.dram_tensor("outT", [F, B], dt.float32, kind="ExternalOutput")

    xdram = [seq, seq_e, seq_t]
    nchunk = t_steps // TC

    with tile.TileContext(nc) as tc:
        with (
            tc.tile_pool(name="const", bufs=1) as constp,
            tc.tile_pool(name="stage", bufs=3) as stagep,
            tc.tile_pool(name="xt", bufs=3) as xtp,
            tc.tile_pool(name="gates", bufs=2) as gatep,
        ):
            # ---------------- weights / constants ----------------
            wi_f32 = constp.tile([128, 3, G], dt.float32)
            nc.sync.dma_start(wi_f32[:], Wi[:].rearrange("(kc k) g -> k kc g", k=128))
            wi_bf = constp.tile([128, 3, G], dt.bfloat16)
            nc.vector.tensor_copy(wi_bf[:], wi_f32[:])

            wh_f32 = constp.tile([128, G], dt.float32)
            nc.sync.dma_start(wh_f32[:], Wh[:])
            wh_bf = constp.tile([128, G], dt.bfloat16)
            nc.vector.tensor_copy(wh_bf[:], wh_f32[:])

            w1_f32 = constp.tile([128, 2, F], dt.float32)
            nc.sync.dma_start(w1_f32[:], W1[:].rearrange("(kc k) m -> k kc m", k=128))
            w1_bf = constp.tile([128, 2, F], dt.bfloat16)
            nc.vector.tensor_copy(w1_bf[:], w1_f32[:])

            w2_f32 = constp.tile([128, F], dt.float32)
            nc.sync.dma_start(w2_f32[:], W2[:])
            w2_bf = constp.tile([128, F], dt.bfloat16)
            nc.vector.tensor_copy(w2_bf[:], w2_f32[:])

            b1t = constp.tile([128, 1], dt.float32)
            nc.sync.dma_start(b1t[:], b1[:].rearrange("(f one) -> f one", one=1))
            b2t = constp.tile([128, 1], dt.float32)
            nc.sync.dma_start(b2t[:], b2[:].rearrange("(f one) -> f one", one=1))

            if not zero_bias:
                # bh folded into z via a rank-1 matmul: u += ones^T @ bh_row.
                bh_row = constp.tile([1, G], dt.bfloat16)
                bh_f32 = constp.tile([1, G], dt.float32)
                nc.sync.dma_start(
                    bh_f32[:], bh[:].rearrange("(one g) -> one g", one=1)
                )
                nc.vector.tensor_copy(bh_row[:], bh_f32[:])
                ones_row = constp.tile([1, B], dt.bfloat16)
                nc.gpsimd.memset(ones_row[:], 1.0)

            # src^T (bf16): cast-DMA then xbar transpose
            src_bm = constp.tile([128, 4, F], dt.bfloat16)
            nc.gpsimd.dma_start(
                src_bm[:], src[:].rearrange("(s p) f -> p s f", p=128)
            )
            srcT = constp.tile([128, 4, 128], dt.bfloat16)
            nc.sync.dma_start_transpose(
                srcT[:], src_bm[:].rearrange("p s f -> p (s f)")
            )

            # ---------------- persistent state (bf16, SBUF) ----------------
            cs = []
            hs = []
            for h in range(2):
                c_h = constp.tile([128, NH], dt.bfloat16, name=f"c_{h}")
                nc.gpsimd.memset(c_h[:], 0.0)
                cs.append(c_h)
                h_h = constp.tile([128, NH], dt.bfloat16, name=f"h_{h}")
                nc.gpsimd.memset(h_h[:], 0.0)
                hs.append(h_h)

            # ---------------- main loop ----------------
            # z ping-pong: [128, 4 quads, 512 b] fp32 = 4 PSUM banks each.
            zp_ctx = tc.tile_pool(name="zp", bufs=2, space="PSUM")
            zp = zp_ctx.__enter__()

            def z_tile(t):
                return zp.tile([128, 4, B], dt.float32, tag="z", name=f"z_{t}")

            def input_proj(z, xts, ts_):
                """u = x_t @ Wi (+ bh) for the full batch, N=512."""
                for qname, qi, woff in QUADS:
                    for kc in range(3):
                        nc.tensor.matmul(
                            z[:, qi, :],
                            wi_bf[:, kc, woff : woff + 128],
                            xts[kc][:, :, ts_, :],
                            start=(kc == 0),
                            stop=False,
                        )
                if not zero_bias:
                    for qname, qi, woff in QUADS:
                        nc.tensor.matmul(
                            z[:, qi, :],
                            bh_row[:, woff : woff + 128],
                            ones_row[:],
                            start=False,
                            stop=False,
                        )

            def recur_half(z, h):
                """Wh accumulate + gates + state update for half h."""
                bs = slice(h * NH, (h + 1) * NH)
                for qname, qi, woff in QUADS:
                    nc.tensor.matmul(
                        z[:, qi, bs],
                        wh_bf[:, woff : woff + 128],
                        hs[h][:],
                        start=False,
                        stop=True,
                    )
                # gates: one sigmoid over [i|f|o], one tanh over g (PSUM src)
                sg = gatep.tile(
                    [128, 3, NH], dt.bfloat16, tag=f"sg{h}", name=f"sg{h}"
                )
                nc.scalar.activation(sg[:], z[:, 0:3, bs], AF.Sigmoid)
                tg = gatep.tile([128, NH], dt.bfloat16, tag=f"tg{h}", name=f"tg{h}")
                nc.scalar.activation(tg[:], z[:, 3, bs], AF.Tanh)

                # cell update (DVE, bf16 2x)
                m2 = gatep.tile([128, NH], dt.bfloat16, tag=f"m2_{h}", name=f"m2{h}")
                nc.vector.tensor_mul(m2[:], sg[:, 0, :], tg[:])
                m1 = gatep.tile([128, NH], dt.bfloat16, tag=f"m1_{h}", name=f"m1{h}")
                nc.vector.tensor_mul(m1[:], sg[:, 1, :], cs[h][:])
                nc.vector.tensor_add(cs[h][:], m1[:], m2[:])

                # h update: tanh(c) on ACT, then DVE mul
                tc2 = gatep.tile([128, NH], dt.bfloat16, tag=f"tc2_{h}", name=f"tc{h}")
                nc.scalar.activation(tc2[:], cs[h][:], AF.Tanh)
                nc.vector.tensor_mul(hs[h][:], sg[:, 2, :], tc2[:])

            cur_xts = None
            cur_z = None
            for ch in range(nchunk):
                t0 = ch * TC
                xts = []
                for name, dram in (("s", seq), ("e", seq_e), ("t", seq_t)):
                    bm = stagep.tile(
                        [128, 4, TC, F],
                        dt.bfloat16,
                        tag=f"bm_{name}",
                        name=f"bm_{name}_{ch}",
                    )
                    nc.gpsimd.dma_start(
                        bm[:],
                        dram[:].rearrange("(s p) t f -> p s t f", p=128)[
                            :, :, t0 : t0 + TC, :
                        ],
                    )
                    xt_ = xtp.tile(
                        [128, 4, TC, 128],
                        dt.bfloat16,
                        tag=f"xt_{name}",
                        name=f"xt_{name}_{ch}",
                    )
                    # out[f, (s,t), bp] = bm[bp, (s,t), f]  (batched 128x128
                    # tile transposes in one xbar instruction)
                    nc.sync.dma_start_transpose(
                        xt_[:], bm[:].rearrange("p s t f -> p (s t f)")
                    )
                    xts.append(xt_)

                for ts_ in range(TC):
                    t = t0 + ts_
                    if t == 0:
                        cur_z = z_tile(0)
                        input_proj(cur_z, xts, 0)
                        cur_xts = xts
                        continue
                    # next step's input projection (independent of h), then
                    # this step's recurrence per half.  Program order sets the
                    # tensor-engine FIFO: the Wh matmuls for half A go first
                    # (critical path), then half the u matmuls run while ACT/
                    # DVE chew on half A, then Wh for half B, then the rest.
                    z = cur_z
                    nz = z_tile(t)
                    recur_half(z, 0)
                    input_proj(nz, xts, ts_)
                    recur_half(z, 1)
                    cur_z = nz
                    cur_xts = xts

            # final step's recurrence
            recur_half(cur_z, 0)
            recur_half(cur_z, 1)

            zp_ctx.__exit__(None, None, None)

            # ---------------- merge layer ----------------
            with tc.tile_pool(name="mp", bufs=1, space="PSUM") as mp:
                ps_hid = mp.tile([128, B], dt.float32)
                for h in range(2):
                    nc.tensor.matmul(
                        ps_hid[:, h * NH : (h + 1) * NH],
                        w1_bf[:, 0, :],
                        hs[h][:],
                        start=True,
                        stop=False,
                    )
                    nc.tensor.matmul(
                        ps_hid[:, h * NH : (h + 1) * NH],
                        w1_bf[:, 1, :],
                        srcT[:, 2 * h : 2 * h + 2, :],
                        start=False,
                        stop=True,
                    )
                hid_bf = constp.tile([128, B], dt.bfloat16)
                nc.scalar.activation(hid_bf[:], ps_hid[:], AF.Relu, bias=b1t[:])

                ps_out = mp.tile([128, B], dt.float32)
                nc.tensor.matmul(ps_out[:], w2_bf[:], hid_bf[:], start=True, stop=True)
                out_sb = constp.tile([128, B], dt.float32)
                nc.scalar.activation(out_sb[:], ps_out[:], AF.Identity, bias=b2t[:])
                nc.sync.dma_start(outT[:], out_sb[:])

    nc.compile()
    return nc


_NC_CACHE: dict = {}


def _get_nc(zero_bias: bool):
    if zero_bias not in _NC_CACHE:
        _NC_CACHE[zero_bias] = build_nc(zero_bias)
    return _NC_CACHE[zero_bias]


def make_in_maps(**inputs):
    """Slice full inputs into per-core input maps (batch data-parallel)."""
    f32 = lambda x: np.ascontiguousarray(np.asarray(x), dtype=np.float32)
    shared = {
        k: f32(inputs[k]) for k in ("Wi", "Wh", "bh", "W1", "b1", "W2", "b2")
    }
    in_maps = []
    for c in range(NCORES):
        sl = slice(c * B, (c + 1) * B)
        m = dict(shared)
        m["seq"] = f32(inputs["seq"][sl])
        m["seq_e"] = f32(inputs["seq_e"][sl])
        m["seq_t"] = f32(inputs["seq_t"][sl])
        m["src"] = f32(inputs["src"][sl])
        in_maps.append(m)
    return in_maps


def kernel(**inputs) -> np.ndarray:
    zero_bias = not np.any(np.asarray(inputs["bh"]))
    nc = _get_nc(zero_bias)
    in_maps = make_in_maps(**inputs)
    res = run_bass_kernel_spmd(nc, in_maps, core_ids=list(range(NCORES)))
    out = np.empty((BFULL, F), np.float32)
    for c in range(NCORES):
        out[c * B : (c + 1) * B] = res.results[c]["outT"].T
    return out
